# revision 8
# baseline (speedup 1.0000x reference)
"""TRN2 Bass kernel for nn_LongTermAttention_18640158064770.

Sharding: 8 cores = (batch b in 0..3) x (query half qh in 0..1).
Key algebra: scores [B,H,Q,NB] is only consumed through scores@w_mu and
scores@w_sigma, so the qry/keys projections and the score matmul collapse to
tiny per-(b,h,q) Gaussian parameters (computed on host in fp64; ~0.1% of
FLOPs). Per core the device computes (fp32 matmuls):
  Bm.T[e,j]   = sum_l k[l,e] Gs[l,j]          (lhsT=k tiles, rhs=Gs tiles)
  vals[j,e']  = sum_e Bm.T[e,j] Wv.T[e,e']    (lhsT=Bm.T, rhs=Wv.T)
  arg.T[j,q]  = A_h(q) + B_h(q) mu_j + C_h(q) mu_j^2   (K=3 matmul)
  r.T[j,q]    = Exp(arg.T)                    (ACT, from PSUM)
  ctx.T[d,q]  = sum_j vals[j,d] r.T[j,q]      (per head, M=64)
  out[q,e'']  = sum_e' ctx.T[e',q] Wo.T[e',e'']
Basis is permuted so sigma groups are contiguous (j-tiles 0,1 -> sigma0;
2,3 -> sigma1); Gs columns and vals rows carry the same permutation, so
ctx = sum_j r_j vals_j is unchanged.
"""
import os
import numpy as np

import concourse.mybir as mybir
import concourse.tile as tile
from concourse import bacc
from concourse.bass_utils import run_bass_kernel_spmd


def _install_ntff_shim():
    """Provide antenv.axon_hooks so trace=True can capture NTFF profiles."""
    try:
        import sys, types
        import antenv
        if hasattr(antenv, "axon_hooks"):
            return
        from trn_agent_boot.trn_boot import _ntff_profile_via_ctypes
        mod = types.ModuleType("antenv.axon_hooks")
        _h = {"hook": _ntff_profile_via_ctypes("/opt/axon/libaxon_pjrt.so")}
        mod.get_axon_ntff_profile_hook = lambda: _h["hook"]
        mod.set_axon_ntff_profile_hook = lambda h: _h.__setitem__("hook", h)
        sys.modules["antenv.axon_hooks"] = mod
        antenv.axon_hooks = mod
    except Exception:
        pass


LAST_EXEC_NS = None

B, L, Q, H, D, NB = 4, 2048, 2048, 16, 64, 512
E = H * D
QC = Q // 2                 # queries per core
P = 128
SIGMAS = np.array([0.005, 0.01])
CLAMP_MIN = 1e-4
DT = (mybir.dt.float32r if os.environ.get("KERNEL_DT") == "f32r"
      else mybir.dt.float32)  # fp32 matmuls: safe precision (output has ~180x
                              # cancellation vs intermediates; bf16 fails)

_NC_CACHE = {}


def _build_nc():
    if "nc" in _NC_CACHE:
        return _NC_CACHE["nc"]
    nc = bacc.Bacc("TRN2", target_bir_lowering=False, debug=False)
    kb = nc.dram_tensor("kb", [L, E], DT, kind="ExternalInput")
    gs = nc.dram_tensor("gs", [L, NB], DT, kind="ExternalInput")
    wvt = nc.dram_tensor("wvt", [E, E], DT, kind="ExternalInput")
    wot = nc.dram_tensor("wot", [E, E], DT, kind="ExternalInput")
    coef = nc.dram_tensor("coef", [2, 48, QC], DT, kind="ExternalInput")
    mkp = nc.dram_tensor("mkp", [3, 4 * P], DT, kind="ExternalInput")
    out = nc.dram_tensor("out", [QC, E], mybir.dt.float32, kind="ExternalOutput")

    ET, LT, JT, QB = E // P, L // P, NB // P, QC // 512
    f32 = mybir.dt.float32

    with tile.TileContext(nc) as tc:
        with (
            tc.tile_pool(name="hold", bufs=1) as hold,
            tc.tile_pool(name="kst", bufs=3) as kst,
            tc.tile_pool(name="gst", bufs=3) as gst,
            tc.tile_pool(name="rp", bufs=6) as rp,
            tc.tile_pool(name="stp", bufs=4) as stp,
            tc.tile_pool(name="cxp", bufs=2) as cxp,
            tc.tile_pool(name="oev", bufs=3) as oev,
            tc.tile_pool(name="ps", bufs=8, space="PSUM") as ps,
        ):
            # ---- persistent SBUF ----
            bmt = hold.tile([P, ET * NB], DT, tag="bmt")     # Bm.T: 8 x [128,512]
            vals = hold.tile([P, JT * E], DT, tag="vals")    # 4 x [128,1024]
            coefs = hold.tile([48, 2 * QC], DT, tag="coefs")  # rows h*3+{A,B,C}
            mks = hold.tile([3, JT * P], DT, tag="mks")      # [1; mu_j; mu_j^2]
            wos = hold.tile([P, ET * E], DT, tag="wos")      # Wo.T: 8 x [128,1024]
            wvs = hold.tile([P, ET * E], DT, tag="wvs")      # Wv.T: 8 x [128,1024]

            nc.sync.dma_start(out=coefs[:, 0:QC], in_=coef[0])
            nc.sync.dma_start(out=coefs[:, QC:2 * QC], in_=coef[1])
            nc.sync.dma_start(out=mks[:], in_=mkp[:])
            for et in range(ET):
                nc.sync.dma_start(out=wos[:, et * E:(et + 1) * E],
                                  in_=wot[et * P:(et + 1) * P, :])
                nc.sync.dma_start(out=wvs[:, et * E:(et + 1) * E],
                                  in_=wvt[et * P:(et + 1) * P, :])

            # ---- phase A: Bm.T[e, j], k/Gs streamed by l-tile ----
            pas = [ps.tile([P, NB], f32, tag="p", name=f"pa{i}") for i in range(ET)]
            for lt in range(LT):
                kt = kst.tile([P, E], DT, tag="kt")
                gt = gst.tile([P, NB], DT, tag="gt")
                nc.sync.dma_start(out=kt[:], in_=kb[lt * P:(lt + 1) * P, :])
                nc.sync.dma_start(out=gt[:], in_=gs[lt * P:(lt + 1) * P, :])
                for et in range(ET):
                    nc.tensor.matmul(pas[et][:], kt[:, et * P:(et + 1) * P],
                                     gt[:], start=(lt == 0), stop=(lt == LT - 1))
            for et in range(ET):
                nc.scalar.copy(bmt[:, et * NB:(et + 1) * NB], pas[et][:])

            # ---- phase B: vals[j, e'] ----
            for jt in range(JT):
                for blk in range(2):
                    pb = ps.tile([P, 512], f32, tag="p")
                    for et in range(ET):
                        nc.tensor.matmul(
                            pb[:],
                            bmt[:, et * NB + jt * P: et * NB + (jt + 1) * P],
                            wvs[:, et * E + blk * 512: et * E + (blk + 1) * 512],
                            start=(et == 0), stop=(et == ET - 1))
                    nc.scalar.copy(vals[:, jt * E + blk * 512: jt * E + (blk + 1) * 512],
                                   pb[:])

            # ---- phase C: per q-block: r, ctx.T, out ----
            for qb in range(QB):
                ctxt = cxp.tile([P, ET * 512], f32, tag="ctxt")  # [E, 512q] slab
                for h in range(H):
                    stg = stp.tile([3, 1024], DT, tag="stg")
                    for s2 in range(2):
                        nc.sync.dma_start(
                            out=stg[:, s2 * 512:(s2 + 1) * 512],
                            in_=coefs[h * 3:h * 3 + 3,
                                      s2 * QC + qb * 512: s2 * QC + (qb + 1) * 512])
                    rts = []
                    for jt in range(JT):
                        s = 0 if jt < 2 else 1
                        pr = ps.tile([P, 512], f32, tag="p")
                        nc.tensor.matmul(
                            pr[:],
                            mks[:, jt * P:(jt + 1) * P],
                            stg[:, s * 512:(s + 1) * 512],
                            start=True, stop=True)
                        rt = rp.tile([P, 512], DT, tag="rt")
                        nc.scalar.activation(rt[:], pr[:],
                                             mybir.ActivationFunctionType.Exp)
                        rts.append(rt)
                    pc_ = ps.tile([64, 512], f32, tag="p")
                    for jt in range(JT):
                        nc.tensor.matmul(
                            pc_[:],
                            vals[:, jt * E + h * D: jt * E + (h + 1) * D],
                            rts[jt][:],
                            start=(jt == 0), stop=(jt == JT - 1))
                    et, off = h // 2, (h % 2) * D
                    nc.scalar.copy(ctxt[off:off + D, et * 512:(et + 1) * 512], pc_[:])
                for qt in range(4):
                    for blk in range(2):
                        po = ps.tile([P, 512], f32, tag="p")
                        for et in range(ET):
                            nc.tensor.matmul(
                                po[:],
                                ctxt[:, et * 512 + qt * P: et * 512 + (qt + 1) * P],
                                wos[:, et * E + blk * 512: et * E + (blk + 1) * 512],
                                start=(et == 0), stop=(et == ET - 1))
                        ot = oev.tile([P, 512], f32, tag="ot")
                        nc.scalar.copy(ot[:], po[:])
                        nc.sync.dma_start(
                            out=out[qb * 512 + qt * P: qb * 512 + (qt + 1) * P,
                                    blk * 512:(blk + 1) * 512],
                            in_=ot[:])
    nc.compile()
    _NC_CACHE["nc"] = nc
    return nc


def _host_prep(k, q, Wq, Wk, w_mu, w_sigma, Gs, basis_mu):
    """fp64 host prep of the tiny Gaussian-parameter path (~0.1% of FLOPs)."""
    f8 = np.float64
    sD = 1.0 / np.sqrt(f8(D))
    perm = np.concatenate([np.arange(0, NB, 2), np.arange(1, NB, 2)])
    mu_p = basis_mu.astype(f8)[perm]
    Gs_p = np.ascontiguousarray(Gs.astype(np.float32)[:, perm])

    g = Gs.astype(f8) @ np.stack([w_mu.astype(f8), w_sigma.astype(f8)], 1)  # [L,2]
    coefs = np.empty((B, 2, 48, Q), np.float32)
    for b in range(B):
        t = k[b].astype(f8).T @ g                                   # [E,2]
        Wh = np.empty((E, H, 2), f8)
        for h in range(H):
            u = Wk.astype(f8)[h * D:(h + 1) * D, :] @ t * sD        # [D,2]
            Wh[:, h, :] = Wq.astype(f8)[h * D:(h + 1) * D, :].T @ u
        sv = np.einsum('qe,ehc->qhc', q[b].astype(f8), Wh)          # [Q,H,2]
        mu = 1.0 / (1.0 + np.exp(-sv[..., 0]))                      # [Q,H]
        sig2 = np.clip(np.logaddexp(0.0, sv[..., 1]), CLAMP_MIN, None)
        for s in range(2):
            var = sig2 + SIGMAS[s] ** 2
            A = -0.5 * mu * mu / var - 0.5 * np.log(2 * np.pi * var)
            Bc = mu / var
            Cc = -0.5 / var
            coefs[b, s, 0::3] = A.T.astype(np.float32)
            coefs[b, s, 1::3] = Bc.T.astype(np.float32)
            coefs[b, s, 2::3] = Cc.T.astype(np.float32)
    mkp = np.stack([np.ones(NB), mu_p, mu_p ** 2]).astype(np.float32)  # [3, 512]
    return Gs_p, coefs, np.ascontiguousarray(mkp)


def kernel(k, q, Wq, Wk, Wv, Wo, w_mu, w_sigma, Gs, basis_mu, basis_sigma):
    k = np.ascontiguousarray(np.asarray(k, np.float32))
    q = np.ascontiguousarray(np.asarray(q, np.float32))
    Gs_p, coefs, mkp = _host_prep(k, q, np.asarray(Wq), np.asarray(Wk),
                                  np.asarray(w_mu), np.asarray(w_sigma),
                                  np.asarray(Gs), np.asarray(basis_mu))
    wvt = np.ascontiguousarray(np.asarray(Wv, np.float32).T)
    wot = np.ascontiguousarray(np.asarray(Wo, np.float32).T)

    nc = _build_nc()
    in_maps = []
    for c in range(8):
        b, qh = c // 2, c % 2
        in_maps.append({
            "kb": k[b], "gs": Gs_p, "wvt": wvt, "wot": wot,
            "coef": np.ascontiguousarray(coefs[b, :, :, qh * QC:(qh + 1) * QC]),
            "mkp": mkp,
        })
    trace = bool(os.environ.get("KERNEL_TRACE"))
    if trace:
        _install_ntff_shim()
    res = run_bass_kernel_spmd(nc, in_maps, list(range(8)), trace=trace)
    global LAST_EXEC_NS
    LAST_EXEC_NS = res.exec_time_ns
    out = np.empty((B, Q, E), np.float32)
    for c in range(8):
        b, qh = c // 2, c % 2
        out[b, qh * QC:(qh + 1) * QC, :] = res.results[c]["out"]
    return out
